# revision 10
# baseline (speedup 1.0000x reference)
"""DecoderRNN (LSTM cell + Bahdanau attention + vocab projection + log-softmax)
on 8 Trainium2 NeuronCores.

Sharding:
  - Embedding lookup + LSTM cell: replicated on every core (full batch N=64).
    The gate GEMMs are moving-operand-bound, so replicating costs no extra PE
    time and avoids gathering h_t before attention.
  - Bahdanau attention: data-parallel over batch (8 rows per core); per-core
    context rows are AllGathered (16 KB).
  - h2o projection: tensor-parallel over vocab (4000 rows per core); local
    log-softmax stats (max, sum-exp) are AllGathered (512 B) and combined
    exactly, each core emits its own [64, 4000] slice of the output.

Weight matrices are passed pre-transposed (contract dim major) so streaming
DMA loads are contiguous; fp32 has no DMA-transpose path on TRN2.
"""
import numpy as np

V, E, H, N, L = 32000, 512, 512, 64, 128
NCORES = 8
NB = N // NCORES        # 8 batch rows per core
VS = V // NCORES        # 4000 vocab rows per core
G4 = 4 * H              # 2048 gates
KC = 2 * H              # 1024 contraction dim of h2o
NKC = KC // 128         # 8 k-chunks
NVC = (VS + 511) // 512  # 8 vocab chunks per core (7x512 + 416)

_cached = None


def _build():
    import contextlib
    import concourse.bass as bass
    import concourse.tile as tile
    from concourse import bacc, mybir
    from concourse.masks import make_identity

    f32 = mybir.dt.float32
    i32 = mybir.dt.int32
    u8 = mybir.dt.uint8
    X = mybir.AxisListType.X
    AF = mybir.ActivationFunctionType
    OP = mybir.AluOpType

    nc = bacc.Bacc("TRN2", target_bir_lowering=False, debug=False,
                   num_devices=NCORES)

    ExtIn = dict(kind="ExternalInput")
    d_ids = nc.dram_tensor("ids", [N, 1], i32, **ExtIn).ap()
    d_h = nc.dram_tensor("h0", [N, H], f32, **ExtIn).ap()
    d_c = nc.dram_tensor("c0", [N, H], f32, **ExtIn).ap()
    d_enc = nc.dram_tensor("enc", [NB * L, H], f32, **ExtIn).ap()
    d_msk = nc.dram_tensor("msk", [NB, L], u8, **ExtIn).ap()
    d_embW = nc.dram_tensor("embW", [V, E], f32, **ExtIn).ap()
    d_wihT = nc.dram_tensor("wihT", [E, G4], f32, **ExtIn).ap()
    d_whhT = nc.dram_tensor("whhT", [H, G4], f32, **ExtIn).ap()
    d_bg = nc.dram_tensor("bg", [1, G4], f32, **ExtIn).ap()      # b_ih + b_hh
    d_awhT = nc.dram_tensor("awhT", [H, H], f32, **ExtIn).ap()   # attn_Wh^T
    d_awsT = nc.dram_tensor("awsT", [H, H], f32, **ExtIn).ap()   # attn_Ws^T
    d_av = nc.dram_tensor("av", [1, H], f32, **ExtIn).ap()
    d_howT = nc.dram_tensor("howT", [KC, VS], f32, **ExtIn).ap()  # h2o_W slice^T
    d_hob = nc.dram_tensor("hob", [1, VS], f32, **ExtIn).ap()
    d_sel = nc.dram_tensor("sel", [N, NB], f32, **ExtIn).ap()    # one-hot row picker

    d_out = nc.dram_tensor("out_c", [N, VS], f32, kind="ExternalOutput").ap()
    d_hto = nc.dram_tensor("ht_o", [N, H], f32, kind="ExternalOutput").ap()
    d_cto = nc.dram_tensor("ct_o", [N, H], f32, kind="ExternalOutput").ap()

    with tile.TileContext(nc) as tc, contextlib.ExitStack() as ctx:
        const = ctx.enter_context(tc.tile_pool(name="const", bufs=1))
        sb = ctx.enter_context(tc.tile_pool(name="sb", bufs=1))
        sb2 = ctx.enter_context(tc.tile_pool(name="sb2", bufs=2))
        lstmw = ctx.enter_context(tc.tile_pool(name="lstmw", bufs=6))
        wstr = ctx.enter_context(tc.tile_pool(name="wstr", bufs=16))
        ppt = ctx.enter_context(tc.tile_pool(name="ppt", bufs=2, space="PSUM"))
        dram = ctx.enter_context(tc.tile_pool(name="dram", bufs=1, space="DRAM"))

        ident = const.tile([128, 128], f32, tag="ident")
        make_identity(nc, ident)
        ones64 = const.tile([1, N], f32, tag="ones64")
        nc.vector.memset(ones64, 1.0)

        def transp(dst_ap, src_ap):
            """PE transpose src [p, f] -> dst [f, p] via PSUM."""
            p, f = src_ap.shape[0], src_ap.shape[-1]
            tp = ppt.tile([128, 128], f32, tag="tp")
            nc.tensor.transpose(out=tp[:f, :p], in_=src_ap, identity=ident[:p, :p])
            nc.vector.tensor_copy(out=dst_ap, in_=tp[:f, :p])

        # ---------------- embedding gather + small input loads ----------------
        ids_sb = const.tile([N, 1], i32, tag="ids")
        nc.sync.dma_start(out=ids_sb, in_=d_ids)
        emb_sb = sb.tile([N, E], f32, tag="emb")
        nc.gpsimd.indirect_dma_start(
            out=emb_sb[:, :], out_offset=None, in_=d_embW,
            in_offset=bass.IndirectOffsetOnAxis(ap=ids_sb[:, :1], axis=0))
        h_sb = sb.tile([N, H], f32, tag="h")
        nc.sync.dma_start(out=h_sb, in_=d_h)
        c_sb = sb.tile([N, H], f32, tag="c")
        nc.sync.dma_start(out=c_sb, in_=d_c)
        bg_sb = const.tile([1, G4], f32, tag="bg")
        nc.sync.dma_start(out=bg_sb, in_=d_bg)

        # enc natural layout [l, b, h] (one 2 MB DMA)
        enc_sb = sb.tile([L, NB, H], f32, tag="enc")
        nc.sync.dma_start(out=enc_sb, in_=d_enc.rearrange("(b l) h -> l b h", l=L))

        # ---------------- LSTM cell (replicated, full batch) ----------------
        embT = sb.tile([128, 4, N], f32, tag="embT")
        for k in range(4):
            transp(embT[:, k, :], emb_sb[:, k * 128:(k + 1) * 128])
        hT = sb.tile([128, 4, N], f32, tag="hT")
        for k in range(4):
            transp(hT[:, k, :], h_sb[:, k * 128:(k + 1) * 128])

        ga = sb.tile([N, 4, H], f32, tag="ga")  # sig_i, sig_f, tanh_g, sig_o
        ph_lstm = contextlib.ExitStack()
        ppg = ph_lstm.enter_context(tc.tile_pool(name="ppg", bufs=2, space="PSUM"))
        for g in range(4):
            gp = ppg.tile([N, H], f32, tag="gp")
            for k in range(4):
                wt = lstmw.tile([128, H], f32, tag="lw")
                nc.sync.dma_start(out=wt, in_=d_wihT[k * 128:(k + 1) * 128,
                                                     g * H:(g + 1) * H])
                nc.tensor.matmul(out=gp, lhsT=embT[:, k, :], rhs=wt,
                                 start=(k == 0), stop=False)
            for k in range(4):
                wt = lstmw.tile([128, H], f32, tag="lw")
                nc.sync.dma_start(out=wt, in_=d_whhT[k * 128:(k + 1) * 128,
                                                     g * H:(g + 1) * H])
                nc.tensor.matmul(out=gp, lhsT=hT[:, k, :], rhs=wt,
                                 start=False, stop=False)
            nc.tensor.matmul(out=gp, lhsT=ones64, rhs=bg_sb[:, g * H:(g + 1) * H],
                             start=False, stop=True)
            nc.scalar.activation(out=ga[:, g, :], in_=gp,
                                 func=AF.Tanh if g == 2 else AF.Sigmoid)

        t1 = sb.tile([N, H], f32, tag="t1")
        nc.vector.tensor_mul(out=t1, in0=ga[:, 1, :], in1=c_sb)        # f*c
        t2 = sb.tile([N, H], f32, tag="t2")
        nc.vector.tensor_mul(out=t2, in0=ga[:, 0, :], in1=ga[:, 2, :])  # i*g
        ct_sb = sb.tile([N, H], f32, tag="ct")
        nc.vector.tensor_add(out=ct_sb, in0=t1, in1=t2)
        tc_sb = sb.tile([N, H], f32, tag="tc")
        nc.scalar.activation(out=tc_sb, in_=ct_sb, func=AF.Tanh)
        ht_sb = sb.tile([N, H], f32, tag="ht")
        nc.vector.tensor_mul(out=ht_sb, in0=ga[:, 3, :], in1=tc_sb)
        nc.sync.dma_start(out=d_cto, in_=ct_sb)
        nc.sync.dma_start(out=d_hto, in_=ht_sb)

        # combT[k,0:4] = h_t^T ; [k,4:8] = context^T (filled after AllGather)
        combT = sb.tile([128, NKC, N], f32, tag="combT")
        for k in range(4):
            transp(combT[:, k, :], ht_sb[:, k * 128:(k + 1) * 128])

        # ---------------- attention (own NB=8 batch rows) ----------------
        # q = h_t @ Wh^T for all 64 rows, then pick own 8 via sel one-hot
        awhT_sb = sb.tile([128, 4, H], f32, tag="awhT")
        nc.sync.dma_start(out=awhT_sb, in_=d_awhT.rearrange("(kc p) n -> p kc n", p=128))
        qp = ppg.tile([N, H], f32, tag="qp")
        for k in range(4):
            nc.tensor.matmul(out=qp, lhsT=combT[:, k, :], rhs=awhT_sb[:, k, :],
                             start=(k == 0), stop=(k == 3))
        q_sb = sb.tile([N, H], f32, tag="q")
        nc.vector.tensor_copy(out=q_sb, in_=qp)
        sel_sb = const.tile([N, NB], f32, tag="sel")
        nc.sync.dma_start(out=sel_sb, in_=d_sel)
        qop = ppg.tile([NB, H], f32, tag="qop")
        nc.tensor.matmul(out=qop, lhsT=sel_sb, rhs=q_sb, start=True, stop=True)
        qo_sb = sb.tile([NB, H], f32, tag="qo")
        nc.vector.tensor_copy(out=qo_sb, in_=qop)
        ph_lstm.close()
        ph_attn = contextlib.ExitStack()
        ppa = ph_attn.enter_context(tc.tile_pool(name="ppa", bufs=2, space="PSUM"))
        pps = ph_attn.enter_context(tc.tile_pool(name="pps", bufs=2, space="PSUM"))
        # q_own^T: [128 h, 4 kc, 8 b] for per-batch tanh bias columns
        qoT = sb.tile([128, 4, NB], f32, tag="qoT")
        for k in range(4):
            transp(qoT[:, k, :], qo_sb[:, k * 128:(k + 1) * 128])

        # enc^T: [128 hin, 4 kc, 8 b, 128 l]
        encT = sb.tile([128, 4, NB, L], f32, tag="encT")
        for b in range(NB):
            for k in range(4):
                transp(encT[:, k, b, :], enc_sb[:, b, k * 128:(k + 1) * 128])

        awsT_sb = sb.tile([128, 4, H], f32, tag="awsT")
        nc.sync.dma_start(out=awsT_sb, in_=d_awsT.rearrange("(kc p) n -> p kc n", p=128))
        avT_sb = const.tile([128, 4], f32, tag="avT")
        nc.sync.dma_start(out=avT_sb, in_=d_av.rearrange("o (k p) -> p (k o)", p=128))

        # proj^T = Ws @ enc^T (+ q bias via per-batch tanh bias), then v-dot.
        # Scores accumulate in PSUM as [1, 4*128] per group of 4 batches.
        sstage = sb.tile([1, NB, L], f32, tag="sstage")
        for nch in range(2):
            scp = pps.tile([1, 4 * L], f32, tag="scp")
            for mh in range(4):
                pp = ppa.tile([128, 4 * L], f32, tag="pp")
                for k in range(4):
                    nc.tensor.matmul(
                        out=pp,
                        lhsT=awsT_sb[:, k, mh * 128:(mh + 1) * 128],
                        rhs=encT[:, k, nch * 4:(nch + 1) * 4, :],
                        start=(k == 0), stop=(k == 3))
                th = sb2.tile([128, 4 * L], f32, tag="th")
                for b in range(4):
                    j = nch * 4 + b
                    nc.scalar.activation(out=th[:, b * L:(b + 1) * L],
                                         in_=pp[:, b * L:(b + 1) * L],
                                         func=AF.Tanh, bias=qoT[:, mh, j:j + 1])
                nc.tensor.matmul(out=scp, lhsT=avT_sb[:, mh:mh + 1], rhs=th,
                                 start=(mh == 0), stop=(mh == 3))
            nc.vector.tensor_copy(out=sstage[0:1, nch * 4:(nch + 1) * 4, :],
                                  in_=scp[0:1, :].rearrange("p (b l) -> p b l", l=L))
        sc_sb = sb.tile([NB, L], f32, tag="sc")
        nc.sync.dma_start(out=sc_sb, in_=sstage)  # partition scatter [1,8,128]->[8,128]

        # mask + softmax over l
        msk_sb = const.tile([NB, L], u8, tag="msk")
        nc.sync.dma_start(out=msk_sb, in_=d_msk)
        mskf = sb.tile([NB, L], f32, tag="mskf")
        nc.vector.tensor_copy(out=mskf, in_=msk_sb)
        nc.scalar.mul(mskf, mskf, -1e30)
        nc.vector.tensor_add(out=sc_sb, in0=sc_sb, in1=mskf)
        amx = sb.tile([NB, 1], f32, tag="amx")
        nc.vector.reduce_max(out=amx, in_=sc_sb, axis=X)
        namx = sb.tile([NB, 1], f32, tag="namx")
        nc.scalar.mul(namx, amx, -1.0)
        aew = sb.tile([NB, L], f32, tag="aew")
        asw = sb.tile([NB, 1], f32, tag="asw")
        nc.scalar.activation(out=aew, in_=sc_sb, func=AF.Exp, bias=namx,
                             accum_out=asw)
        arw = sb.tile([NB, 1], f32, tag="arw")
        nc.vector.reciprocal(out=arw, in_=asw)
        aw_sb = sb.tile([NB, L], f32, tag="aw")
        nc.vector.tensor_scalar_mul(aw_sb, aew, arw)

        # attn_w^T [128 l, 8 b], then context rows via M=1 matmuls
        awt = sb.tile([L, NB], f32, tag="awt")
        transp(awt, aw_sb)
        cstage = sb.tile([1, NB, H], f32, tag="cstage")
        for j in range(NB):
            cp = pps.tile([1, H], f32, tag="cp")
            nc.tensor.matmul(out=cp, lhsT=awt[:, j:j + 1], rhs=enc_sb[:, j, :],
                             start=True, stop=True)
            nc.vector.tensor_copy(out=cstage[0:1, j, :], in_=cp[0:1, :])
        ph_attn.close()
        ppm = ctx.enter_context(tc.tile_pool(name="ppm", bufs=4, space="PSUM"))

        # ---------------- AllGather context ----------------
        cb_in = dram.tile([NB, H], f32, tag="cbi")
        nc.sync.dma_start(out=cb_in, in_=cstage)
        cb_out = dram.tile([N, H], f32, tag="cbo")
        nc.gpsimd.collective_compute(
            "AllGather", OP.bypass, replica_groups=[list(range(NCORES))],
            ins=[cb_in[:, :].opt()], outs=[cb_out[:, :].opt()])
        ctxg = sb.tile([N, H], f32, tag="ctxg")
        nc.sync.dma_start(out=ctxg, in_=cb_out)
        for k in range(4):
            transp(combT[:, 4 + k, :], ctxg[:, k * 128:(k + 1) * 128])

        # ---------------- h2o projection (vocab-sharded) ----------------
        hob_sb = sb.tile([1, VS], f32, tag="hob")
        nc.sync.dma_start(out=hob_sb, in_=d_hob)
        logits = sb.tile([N, VS], f32, tag="logits")
        rmx8 = sb.tile([N, NVC], f32, tag="rmx8")
        for n in range(NVC):
            cs = min(512, VS - n * 512)
            pmt = ppm.tile([N, 512], f32, tag="pmt")
            for k in range(NKC):
                wt = wstr.tile([128, 512], f32, tag="wt")
                nc.sync.dma_start(out=wt[:, :cs],
                                  in_=d_howT[k * 128:(k + 1) * 128,
                                             n * 512:n * 512 + cs])
                nc.tensor.matmul(out=pmt[:, :cs], lhsT=combT[:, k, :],
                                 rhs=wt[:, :cs], start=(k == 0), stop=False)
            nc.tensor.matmul(out=pmt[:, :cs], lhsT=ones64,
                             rhs=hob_sb[:, n * 512:n * 512 + cs],
                             start=False, stop=True)
            nc.vector.tensor_copy(out=logits[:, n * 512:n * 512 + cs],
                                  in_=pmt[:, :cs])
            nc.vector.reduce_max(out=rmx8[:, n:n + 1], in_=pmt[:, :cs], axis=X)

        # local log-softmax stats
        mloc = sb.tile([N, 1], f32, tag="mloc")
        nc.vector.reduce_max(out=mloc, in_=rmx8, axis=X)
        nmloc = sb.tile([N, 1], f32, tag="nmloc")
        nc.scalar.mul(nmloc, mloc, -1.0)
        sacc = sb.tile([N, NVC], f32, tag="sacc")
        for n in range(NVC):
            cs = min(512, VS - n * 512)
            junk = sb2.tile([N, 512], f32, tag="junk")
            nc.scalar.activation(out=junk[:, :cs], in_=logits[:, n * 512:n * 512 + cs],
                                 func=AF.Exp, bias=nmloc,
                                 accum_out=sacc[:, n:n + 1])
        sloc = sb.tile([N, 1], f32, tag="sloc")
        nc.vector.reduce_sum(out=sloc, in_=sacc, axis=X)

        # ---------------- AllGather (max, sumexp) stats ----------------
        st_sb = sb.tile([N, 2], f32, tag="st")
        nc.vector.tensor_copy(out=st_sb[:, 0:1], in_=mloc)
        nc.vector.tensor_copy(out=st_sb[:, 1:2], in_=sloc)
        stb_in = dram.tile([N, 2], f32, tag="sbi")
        nc.sync.dma_start(out=stb_in, in_=st_sb)
        stb_out = dram.tile([NCORES * N, 2], f32, tag="sbo")
        nc.gpsimd.collective_compute(
            "AllGather", OP.bypass, replica_groups=[list(range(NCORES))],
            ins=[stb_in[:, :].opt()], outs=[stb_out[:, :].opt()])
        sall = sb.tile([N, NCORES, 2], f32, tag="sall")
        nc.sync.dma_start(out=sall,
                          in_=stb_out[:, :].rearrange("(r n) s -> n r s", n=N))
        mg = sb.tile([N, 1], f32, tag="mg")
        nc.vector.reduce_max(out=mg, in_=sall[:, :, 0], axis=X)
        dmm = sb.tile([N, NCORES], f32, tag="dmm")
        nc.vector.tensor_tensor(out=dmm, in0=sall[:, :, 0],
                                in1=mg.to_broadcast([N, NCORES]), op=OP.subtract)
        edm = sb.tile([N, NCORES], f32, tag="edm")
        nc.scalar.activation(out=edm, in_=dmm, func=AF.Exp)
        wse = sb.tile([N, NCORES], f32, tag="wse")
        nc.vector.tensor_mul(out=wse, in0=edm, in1=sall[:, :, 1])
        sg = sb.tile([N, 1], f32, tag="sg")
        nc.vector.reduce_sum(out=sg, in_=wse, axis=X)
        lsg = sb.tile([N, 1], f32, tag="lsg")
        nc.scalar.activation(out=lsg, in_=sg, func=AF.Ln)
        off = sb.tile([N, 1], f32, tag="off")
        nc.vector.tensor_add(out=off, in0=mg, in1=lsg)
        noff = sb.tile([N, 1], f32, tag="noff")
        nc.scalar.mul(noff, off, -1.0)

        # final: out = logits - (Mg + log Sg), in place
        nc.scalar.activation(out=logits, in_=logits, func=AF.Identity, bias=noff,
                             scale=1.0)
        nc.sync.dma_start(out=d_out, in_=logits)

    nc.compile()
    return nc


def _get_nc():
    global _cached
    if _cached is None:
        _cached = _build()
    return _cached


def _make_in_maps(input_ids, h, c, encoder_hiddens, attn_mask, embed_W,
                  w_ih, b_ih, w_hh, b_hh, attn_Wh, attn_Ws, attn_v,
                  h2o_W, h2o_b):
    f = lambda a: np.ascontiguousarray(np.asarray(a, dtype=np.float32))

    ids32 = np.asarray(input_ids).astype(np.int32).reshape(N, 1)
    h = f(h); c = f(c)
    enc = f(encoder_hiddens)                   # [N, L, H]
    msk = np.asarray(attn_mask).astype(np.uint8)
    embW = f(embed_W)
    wihT = np.ascontiguousarray(f(w_ih).T)     # [E, 4H]
    whhT = np.ascontiguousarray(f(w_hh).T)     # [H, 4H]
    bg = (np.asarray(b_ih, np.float64) + np.asarray(b_hh, np.float64)) \
        .astype(np.float32).reshape(1, G4)
    awhT = np.ascontiguousarray(f(attn_Wh).T)
    awsT = np.ascontiguousarray(f(attn_Ws).T)
    av = f(attn_v).reshape(1, H)
    hoW = f(h2o_W)
    hob = f(h2o_b)

    in_maps = []
    for k in range(NCORES):
        sel = np.zeros((N, NB), np.float32)
        sel[np.arange(k * NB, (k + 1) * NB), np.arange(NB)] = 1.0
        in_maps.append({
            "ids": ids32,
            "h0": h,
            "c0": c,
            "enc": np.ascontiguousarray(
                enc[k * NB:(k + 1) * NB].reshape(NB * L, H)),
            "msk": np.ascontiguousarray(msk[k * NB:(k + 1) * NB]),
            "embW": embW,
            "wihT": wihT,
            "whhT": whhT,
            "bg": bg,
            "awhT": awhT,
            "awsT": awsT,
            "av": av,
            "howT": np.ascontiguousarray(hoW[k * VS:(k + 1) * VS].T),
            "hob": np.ascontiguousarray(hob[k * VS:(k + 1) * VS]).reshape(1, VS),
            "sel": sel,
        })
    return in_maps


def kernel(**inputs):
    from concourse.bass_utils import run_bass_kernel_spmd

    nc = _get_nc()
    in_maps = _make_in_maps(**inputs)
    res = run_bass_kernel_spmd(nc, in_maps, list(range(NCORES)))
    out = np.concatenate([res.results[k]["out_c"] for k in range(NCORES)], axis=1)
    ht = res.results[0]["ht_o"]
    ct = res.results[0]["ct_o"]
    return out, ht, ct


# revision 13
# speedup vs baseline: 1.0015x; 1.0015x over previous
"""DecoderRNN (LSTM cell + Bahdanau attention + vocab projection + log-softmax)
on 8 Trainium2 NeuronCores.

Sharding:
  - Embedding lookup + LSTM cell: replicated on every core (full batch N=64).
    The gate GEMMs are moving-operand-bound, so replicating costs no extra PE
    time and avoids gathering h_t before attention.
  - Bahdanau attention: data-parallel over batch (8 rows per core); per-core
    context rows are AllGathered (16 KB).
  - h2o projection: tensor-parallel over vocab (4000 rows per core); local
    log-softmax stats (max, sum-exp) are AllGathered (512 B) and combined
    exactly, each core emits its own [64, 4000] slice of the output.

Weight matrices are passed pre-transposed (contract dim major) so streaming
DMA loads are contiguous; fp32 has no DMA-transpose path on TRN2.
"""
import numpy as np

V, E, H, N, L = 32000, 512, 512, 64, 128
NCORES = 8
NB = N // NCORES        # 8 batch rows per core
VS = V // NCORES        # 4000 vocab rows per core
G4 = 4 * H              # 2048 gates
KC = 2 * H              # 1024 contraction dim of h2o
NKC = KC // 128         # 8 k-chunks
NVC = (VS + 511) // 512  # 8 vocab chunks per core (7x512 + 416)

_cached = None


def _build():
    import contextlib
    import concourse.bass as bass
    import concourse.tile as tile
    from concourse import bacc, mybir
    from concourse.masks import make_identity

    f32 = mybir.dt.float32
    i32 = mybir.dt.int32
    u8 = mybir.dt.uint8
    X = mybir.AxisListType.X
    AF = mybir.ActivationFunctionType
    OP = mybir.AluOpType

    nc = bacc.Bacc("TRN2", target_bir_lowering=False, debug=False,
                   num_devices=NCORES)

    ExtIn = dict(kind="ExternalInput")
    d_ids = nc.dram_tensor("ids", [N, 1], i32, **ExtIn).ap()
    d_h = nc.dram_tensor("h0", [N, H], f32, **ExtIn).ap()
    d_c = nc.dram_tensor("c0", [N, H], f32, **ExtIn).ap()
    d_enc = nc.dram_tensor("enc", [NB * L, H], f32, **ExtIn).ap()
    d_msk = nc.dram_tensor("msk", [NB, L], u8, **ExtIn).ap()
    d_embW = nc.dram_tensor("embW", [V, E], f32, **ExtIn).ap()
    d_wihT = nc.dram_tensor("wihT", [E, G4], f32, **ExtIn).ap()
    d_whhT = nc.dram_tensor("whhT", [H, G4], f32, **ExtIn).ap()
    d_bg = nc.dram_tensor("bg", [1, G4], f32, **ExtIn).ap()      # b_ih + b_hh
    d_awhT = nc.dram_tensor("awhT", [H, H], f32, **ExtIn).ap()   # attn_Wh^T
    d_awsT = nc.dram_tensor("awsT", [H, H], f32, **ExtIn).ap()   # attn_Ws^T
    d_av = nc.dram_tensor("av", [1, H], f32, **ExtIn).ap()
    d_howT = nc.dram_tensor("howT", [KC, VS], f32, **ExtIn).ap()  # h2o_W slice^T
    d_hob = nc.dram_tensor("hob", [1, VS], f32, **ExtIn).ap()
    d_sel = nc.dram_tensor("sel", [N, NB], f32, **ExtIn).ap()    # one-hot row picker

    d_out = nc.dram_tensor("out_c", [N, VS], f32, kind="ExternalOutput").ap()
    d_hto = nc.dram_tensor("ht_o", [N, H], f32, kind="ExternalOutput").ap()
    d_cto = nc.dram_tensor("ct_o", [N, H], f32, kind="ExternalOutput").ap()

    with tile.TileContext(nc) as tc, contextlib.ExitStack() as ctx:
        const = ctx.enter_context(tc.tile_pool(name="const", bufs=1))
        sb = ctx.enter_context(tc.tile_pool(name="sb", bufs=1))
        sb2 = ctx.enter_context(tc.tile_pool(name="sb2", bufs=2))
        lstmw = ctx.enter_context(tc.tile_pool(name="lstmw", bufs=4))
        wstr = ctx.enter_context(tc.tile_pool(name="wstr", bufs=23))
        ppt = ctx.enter_context(tc.tile_pool(name="ppt", bufs=2, space="PSUM"))
        dram = ctx.enter_context(tc.tile_pool(name="dram", bufs=1, space="DRAM"))

        ident = const.tile([128, 128], f32, tag="ident")
        make_identity(nc, ident)
        ones64 = const.tile([1, N], f32, tag="ones64")
        nc.vector.memset(ones64, 1.0)

        def transp(dst_ap, src_ap):
            """PE transpose src [p, f] -> dst [f, p] via PSUM."""
            p, f = src_ap.shape[0], src_ap.shape[-1]
            tp = ppt.tile([128, 128], f32, tag="tp")
            nc.tensor.transpose(out=tp[:f, :p], in_=src_ap, identity=ident[:p, :p])
            nc.vector.tensor_copy(out=dst_ap, in_=tp[:f, :p])

        # ---------------- embedding gather + small input loads ----------------
        ids_sb = const.tile([N, 1], i32, tag="ids")
        nc.sync.dma_start(out=ids_sb, in_=d_ids)
        emb_sb = sb.tile([N, E], f32, tag="emb")
        nc.gpsimd.indirect_dma_start(
            out=emb_sb[:, :], out_offset=None, in_=d_embW,
            in_offset=bass.IndirectOffsetOnAxis(ap=ids_sb[:, :1], axis=0))
        h_sb = sb.tile([N, H], f32, tag="h")
        nc.sync.dma_start(out=h_sb, in_=d_h)
        c_sb = sb.tile([N, H], f32, tag="c")
        nc.sync.dma_start(out=c_sb, in_=d_c)
        bg_sb = const.tile([1, G4], f32, tag="bg")
        nc.sync.dma_start(out=bg_sb, in_=d_bg)

        # enc natural layout [l, b, h] (one 2 MB DMA)
        enc_sb = sb.tile([L, NB, H], f32, tag="enc")
        nc.sync.dma_start(out=enc_sb, in_=d_enc.rearrange("(b l) h -> l b h", l=L))

        # ---------------- LSTM cell (replicated, full batch) ----------------
        embT = sb.tile([128, 4, N], f32, tag="embT")
        for k in range(4):
            transp(embT[:, k, :], emb_sb[:, k * 128:(k + 1) * 128])
        hT = sb.tile([128, 4, N], f32, tag="hT")
        for k in range(4):
            transp(hT[:, k, :], h_sb[:, k * 128:(k + 1) * 128])

        ga = sb.tile([N, 4, H], f32, tag="ga")  # sig_i, sig_f, tanh_g, sig_o
        ph_lstm = contextlib.ExitStack()
        ppg = ph_lstm.enter_context(tc.tile_pool(name="ppg", bufs=2, space="PSUM"))
        for g in range(4):
            gp = ppg.tile([N, H], f32, tag="gp")
            for k in range(4):
                wt = lstmw.tile([128, H], f32, tag="lw")
                nc.sync.dma_start(out=wt, in_=d_wihT[k * 128:(k + 1) * 128,
                                                     g * H:(g + 1) * H])
                nc.tensor.matmul(out=gp, lhsT=embT[:, k, :], rhs=wt,
                                 start=(k == 0), stop=False)
            for k in range(4):
                wt = lstmw.tile([128, H], f32, tag="lw")
                nc.sync.dma_start(out=wt, in_=d_whhT[k * 128:(k + 1) * 128,
                                                     g * H:(g + 1) * H])
                nc.tensor.matmul(out=gp, lhsT=hT[:, k, :], rhs=wt,
                                 start=False, stop=False)
            nc.tensor.matmul(out=gp, lhsT=ones64, rhs=bg_sb[:, g * H:(g + 1) * H],
                             start=False, stop=True)
            nc.scalar.activation(out=ga[:, g, :], in_=gp,
                                 func=AF.Tanh if g == 2 else AF.Sigmoid)

        t1 = sb.tile([N, H], f32, tag="t1")
        nc.vector.tensor_mul(out=t1, in0=ga[:, 1, :], in1=c_sb)        # f*c
        t2 = sb.tile([N, H], f32, tag="t2")
        nc.vector.tensor_mul(out=t2, in0=ga[:, 0, :], in1=ga[:, 2, :])  # i*g
        ct_sb = sb.tile([N, H], f32, tag="ct")
        nc.vector.tensor_add(out=ct_sb, in0=t1, in1=t2)
        tc_sb = sb.tile([N, H], f32, tag="tc")
        nc.scalar.activation(out=tc_sb, in_=ct_sb, func=AF.Tanh)
        ht_sb = sb.tile([N, H], f32, tag="ht")
        nc.vector.tensor_mul(out=ht_sb, in0=ga[:, 3, :], in1=tc_sb)
        nc.sync.dma_start(out=d_cto, in_=ct_sb)
        nc.sync.dma_start(out=d_hto, in_=ht_sb)

        # combT[k,0:4] = h_t^T ; [k,4:8] = context^T (filled after AllGather)
        combT = sb.tile([128, NKC, N], f32, tag="combT")
        for k in range(4):
            transp(combT[:, k, :], ht_sb[:, k * 128:(k + 1) * 128])

        # ---------------- attention (own NB=8 batch rows) ----------------
        # q = h_t @ Wh^T for all 64 rows, then pick own 8 via sel one-hot
        awhT_sb = sb.tile([128, 4, H], f32, tag="awhT")
        nc.sync.dma_start(out=awhT_sb, in_=d_awhT.rearrange("(kc p) n -> p kc n", p=128))
        qp = ppg.tile([N, H], f32, tag="qp")
        for k in range(4):
            nc.tensor.matmul(out=qp, lhsT=combT[:, k, :], rhs=awhT_sb[:, k, :],
                             start=(k == 0), stop=(k == 3))
        q_sb = sb.tile([N, H], f32, tag="q")
        nc.vector.tensor_copy(out=q_sb, in_=qp)
        sel_sb = const.tile([N, NB], f32, tag="sel")
        nc.sync.dma_start(out=sel_sb, in_=d_sel)
        qop = ppg.tile([NB, H], f32, tag="qop")
        nc.tensor.matmul(out=qop, lhsT=sel_sb, rhs=q_sb, start=True, stop=True)
        qo_sb = sb.tile([NB, H], f32, tag="qo")
        nc.vector.tensor_copy(out=qo_sb, in_=qop)
        ph_lstm.close()
        ph_attn = contextlib.ExitStack()
        ppa = ph_attn.enter_context(tc.tile_pool(name="ppa", bufs=2, space="PSUM"))
        pps = ph_attn.enter_context(tc.tile_pool(name="pps", bufs=2, space="PSUM"))
        # q_own^T: [128 h, 4 kc, 8 b] for per-batch tanh bias columns
        qoT = sb.tile([128, 4, NB], f32, tag="qoT")
        for k in range(4):
            transp(qoT[:, k, :], qo_sb[:, k * 128:(k + 1) * 128])

        # enc^T: [128 hin, 4 kc, 8 b, 128 l]
        encT = sb.tile([128, 4, NB, L], f32, tag="encT")
        for b in range(NB):
            for k in range(4):
                transp(encT[:, k, b, :], enc_sb[:, b, k * 128:(k + 1) * 128])

        awsT_sb = sb.tile([128, 4, H], f32, tag="awsT")
        nc.sync.dma_start(out=awsT_sb, in_=d_awsT.rearrange("(kc p) n -> p kc n", p=128))
        avT_sb = const.tile([128, 4], f32, tag="avT")
        nc.sync.dma_start(out=avT_sb, in_=d_av.rearrange("o (k p) -> p (k o)", p=128))

        # proj^T = Ws @ enc^T (+ q bias via per-batch tanh bias), then v-dot.
        # Scores accumulate in PSUM as [1, 4*128] per group of 4 batches.
        sstage = sb.tile([1, NB, L], f32, tag="sstage")
        for nch in range(2):
            scp = pps.tile([1, 4 * L], f32, tag="scp")
            for mh in range(4):
                pp = ppa.tile([128, 4 * L], f32, tag="pp")
                for k in range(4):
                    nc.tensor.matmul(
                        out=pp,
                        lhsT=awsT_sb[:, k, mh * 128:(mh + 1) * 128],
                        rhs=encT[:, k, nch * 4:(nch + 1) * 4, :],
                        start=(k == 0), stop=(k == 3))
                th = sb2.tile([128, 4 * L], f32, tag="th")
                for b in range(4):
                    j = nch * 4 + b
                    nc.scalar.activation(out=th[:, b * L:(b + 1) * L],
                                         in_=pp[:, b * L:(b + 1) * L],
                                         func=AF.Tanh, bias=qoT[:, mh, j:j + 1])
                nc.tensor.matmul(out=scp, lhsT=avT_sb[:, mh:mh + 1], rhs=th,
                                 start=(mh == 0), stop=(mh == 3))
            nc.vector.tensor_copy(out=sstage[0:1, nch * 4:(nch + 1) * 4, :],
                                  in_=scp[0:1, :].rearrange("p (b l) -> p b l", l=L))
        sc_sb = sb.tile([NB, L], f32, tag="sc")
        nc.sync.dma_start(out=sc_sb, in_=sstage)  # partition scatter [1,8,128]->[8,128]

        # mask + softmax over l
        msk_sb = const.tile([NB, L], u8, tag="msk")
        nc.sync.dma_start(out=msk_sb, in_=d_msk)
        mskf = sb.tile([NB, L], f32, tag="mskf")
        nc.vector.tensor_copy(out=mskf, in_=msk_sb)
        nc.scalar.mul(mskf, mskf, -1e30)
        nc.vector.tensor_add(out=sc_sb, in0=sc_sb, in1=mskf)
        amx = sb.tile([NB, 1], f32, tag="amx")
        nc.vector.reduce_max(out=amx, in_=sc_sb, axis=X)
        namx = sb.tile([NB, 1], f32, tag="namx")
        nc.scalar.mul(namx, amx, -1.0)
        aew = sb.tile([NB, L], f32, tag="aew")
        asw = sb.tile([NB, 1], f32, tag="asw")
        nc.scalar.activation(out=aew, in_=sc_sb, func=AF.Exp, bias=namx,
                             accum_out=asw)
        arw = sb.tile([NB, 1], f32, tag="arw")
        nc.vector.reciprocal(out=arw, in_=asw)
        aw_sb = sb.tile([NB, L], f32, tag="aw")
        nc.vector.tensor_scalar_mul(aw_sb, aew, arw)

        # attn_w^T [128 l, 8 b], then context rows via M=1 matmuls
        awt = sb.tile([L, NB], f32, tag="awt")
        transp(awt, aw_sb)
        cstage = sb.tile([1, NB, H], f32, tag="cstage")
        for j in range(NB):
            cp = pps.tile([1, H], f32, tag="cp")
            nc.tensor.matmul(out=cp, lhsT=awt[:, j:j + 1], rhs=enc_sb[:, j, :],
                             start=True, stop=True)
            nc.vector.tensor_copy(out=cstage[0:1, j, :], in_=cp[0:1, :])
        ph_attn.close()
        ppm = ctx.enter_context(tc.tile_pool(name="ppm", bufs=4, space="PSUM"))

        # ---------------- h2o projection phase A: h_t half ----------------
        # Runs while the context AllGather is in flight. Partial sums land in
        # the logits SBUF tile; phase B adds the context half from PSUM.
        hob_sb = sb.tile([1, VS], f32, tag="hob")
        nc.sync.dma_start(out=hob_sb, in_=d_hob)
        logits = sb.tile([N, VS], f32, tag="logits")
        for n in range(NVC):
            cs = min(512, VS - n * 512)
            pmt = ppm.tile([N, 512], f32, tag="pmt")
            for k in range(4):
                wt = wstr.tile([128, 512], f32, tag="wt")
                nc.sync.dma_start(out=wt[:, :cs],
                                  in_=d_howT[k * 128:(k + 1) * 128,
                                             n * 512:n * 512 + cs])
                nc.tensor.matmul(out=pmt[:, :cs], lhsT=combT[:, k, :],
                                 rhs=wt[:, :cs], start=(k == 0), stop=False)
            nc.tensor.matmul(out=pmt[:, :cs], lhsT=ones64,
                             rhs=hob_sb[:, n * 512:n * 512 + cs],
                             start=False, stop=True)
            nc.vector.tensor_copy(out=logits[:, n * 512:n * 512 + cs],
                                  in_=pmt[:, :cs])

        # ---------------- AllGather context ----------------
        cb_in = dram.tile([NB, H], f32, tag="cbi")
        nc.sync.dma_start(out=cb_in, in_=cstage)
        cb_out = dram.tile([N, H], f32, tag="cbo")
        nc.gpsimd.collective_compute(
            "AllGather", OP.bypass, replica_groups=[list(range(NCORES))],
            ins=[cb_in[:, :].opt()], outs=[cb_out[:, :].opt()])
        ctxg = sb.tile([N, H], f32, tag="ctxg")
        nc.sync.dma_start(out=ctxg, in_=cb_out)
        for k in range(4):
            transp(combT[:, 4 + k, :], ctxg[:, k * 128:(k + 1) * 128])

        # ---------------- phase B: context half + per-chunk stats ----------
        rmx8 = sb.tile([N, NVC], f32, tag="rmx8")
        nrmx8 = sb.tile([N, NVC], f32, tag="nrmx8")
        sacc = sb.tile([N, NVC], f32, tag="sacc")
        for n in range(NVC):
            cs = min(512, VS - n * 512)
            nsl = slice(n * 512, n * 512 + cs)
            pmt = ppm.tile([N, 512], f32, tag="pmt")
            for k in range(4, NKC):
                wt = wstr.tile([128, 512], f32, tag="wt")
                nc.sync.dma_start(out=wt[:, :cs],
                                  in_=d_howT[k * 128:(k + 1) * 128,
                                             n * 512:n * 512 + cs])
                nc.tensor.matmul(out=pmt[:, :cs], lhsT=combT[:, k, :],
                                 rhs=wt[:, :cs], start=(k == 4), stop=(k == NKC - 1))
            nc.vector.tensor_add(out=logits[:, nsl], in0=logits[:, nsl],
                                 in1=pmt[:, :cs])
            nc.vector.reduce_max(out=rmx8[:, n:n + 1], in_=logits[:, nsl], axis=X)
            nc.vector.tensor_scalar_mul(nrmx8[:, n:n + 1], rmx8[:, n:n + 1], -1.0)
            junk = sb2.tile([N, 512], f32, tag="junk")
            nc.scalar.activation(out=junk[:, :cs], in_=logits[:, nsl],
                                 func=AF.Exp, bias=nrmx8[:, n:n + 1],
                                 accum_out=sacc[:, n:n + 1])

        # combine per-chunk stats: m_loc = max_n m_n; S_loc = sum s_n*e^(m_n-m_loc)
        mloc = sb.tile([N, 1], f32, tag="mloc")
        nc.vector.reduce_max(out=mloc, in_=rmx8, axis=X)
        dmn = sb.tile([N, NVC], f32, tag="dmn")
        nc.vector.tensor_tensor(out=dmn, in0=rmx8,
                                in1=mloc.to_broadcast([N, NVC]), op=OP.subtract)
        edmn = sb.tile([N, NVC], f32, tag="edmn")
        nc.scalar.activation(out=edmn, in_=dmn, func=AF.Exp)
        wsn = sb.tile([N, NVC], f32, tag="wsn")
        nc.vector.tensor_mul(out=wsn, in0=edmn, in1=sacc)
        sloc = sb.tile([N, 1], f32, tag="sloc")
        nc.vector.reduce_sum(out=sloc, in_=wsn, axis=X)

        # ---------------- AllGather (max, sumexp) stats ----------------
        st_sb = sb.tile([N, 2], f32, tag="st")
        nc.vector.tensor_copy(out=st_sb[:, 0:1], in_=mloc)
        nc.vector.tensor_copy(out=st_sb[:, 1:2], in_=sloc)
        stb_in = dram.tile([N, 2], f32, tag="sbi")
        nc.sync.dma_start(out=stb_in, in_=st_sb)
        stb_out = dram.tile([NCORES * N, 2], f32, tag="sbo")
        nc.gpsimd.collective_compute(
            "AllGather", OP.bypass, replica_groups=[list(range(NCORES))],
            ins=[stb_in[:, :].opt()], outs=[stb_out[:, :].opt()])
        sall = sb.tile([N, NCORES, 2], f32, tag="sall")
        nc.sync.dma_start(out=sall,
                          in_=stb_out[:, :].rearrange("(r n) s -> n r s", n=N))
        mg = sb.tile([N, 1], f32, tag="mg")
        nc.vector.reduce_max(out=mg, in_=sall[:, :, 0], axis=X)
        dmm = sb.tile([N, NCORES], f32, tag="dmm")
        nc.vector.tensor_tensor(out=dmm, in0=sall[:, :, 0],
                                in1=mg.to_broadcast([N, NCORES]), op=OP.subtract)
        edm = sb.tile([N, NCORES], f32, tag="edm")
        nc.scalar.activation(out=edm, in_=dmm, func=AF.Exp)
        wse = sb.tile([N, NCORES], f32, tag="wse")
        nc.vector.tensor_mul(out=wse, in0=edm, in1=sall[:, :, 1])
        sg = sb.tile([N, 1], f32, tag="sg")
        nc.vector.reduce_sum(out=sg, in_=wse, axis=X)
        lsg = sb.tile([N, 1], f32, tag="lsg")
        nc.scalar.activation(out=lsg, in_=sg, func=AF.Ln)
        off = sb.tile([N, 1], f32, tag="off")
        nc.vector.tensor_add(out=off, in0=mg, in1=lsg)
        noff = sb.tile([N, 1], f32, tag="noff")
        nc.scalar.mul(noff, off, -1.0)

        # final: out = logits - (Mg + log Sg), in place
        nc.scalar.activation(out=logits, in_=logits, func=AF.Identity, bias=noff,
                             scale=1.0)
        nc.sync.dma_start(out=d_out, in_=logits)

    nc.compile()
    return nc


def _get_nc():
    global _cached
    if _cached is None:
        _cached = _build()
    return _cached


def _make_in_maps(input_ids, h, c, encoder_hiddens, attn_mask, embed_W,
                  w_ih, b_ih, w_hh, b_hh, attn_Wh, attn_Ws, attn_v,
                  h2o_W, h2o_b):
    f = lambda a: np.ascontiguousarray(np.asarray(a, dtype=np.float32))

    ids32 = np.asarray(input_ids).astype(np.int32).reshape(N, 1)
    h = f(h); c = f(c)
    enc = f(encoder_hiddens)                   # [N, L, H]
    msk = np.asarray(attn_mask).astype(np.uint8)
    embW = f(embed_W)
    wihT = np.ascontiguousarray(f(w_ih).T)     # [E, 4H]
    whhT = np.ascontiguousarray(f(w_hh).T)     # [H, 4H]
    bg = (np.asarray(b_ih, np.float64) + np.asarray(b_hh, np.float64)) \
        .astype(np.float32).reshape(1, G4)
    awhT = np.ascontiguousarray(f(attn_Wh).T)
    awsT = np.ascontiguousarray(f(attn_Ws).T)
    av = f(attn_v).reshape(1, H)
    hoW = f(h2o_W)
    hob = f(h2o_b)

    in_maps = []
    for k in range(NCORES):
        sel = np.zeros((N, NB), np.float32)
        sel[np.arange(k * NB, (k + 1) * NB), np.arange(NB)] = 1.0
        in_maps.append({
            "ids": ids32,
            "h0": h,
            "c0": c,
            "enc": np.ascontiguousarray(
                enc[k * NB:(k + 1) * NB].reshape(NB * L, H)),
            "msk": np.ascontiguousarray(msk[k * NB:(k + 1) * NB]),
            "embW": embW,
            "wihT": wihT,
            "whhT": whhT,
            "bg": bg,
            "awhT": awhT,
            "awsT": awsT,
            "av": av,
            "howT": np.ascontiguousarray(hoW[k * VS:(k + 1) * VS].T),
            "hob": np.ascontiguousarray(hob[k * VS:(k + 1) * VS]).reshape(1, VS),
            "sel": sel,
        })
    return in_maps


def kernel(**inputs):
    from concourse.bass_utils import run_bass_kernel_spmd

    nc = _get_nc()
    in_maps = _make_in_maps(**inputs)
    res = run_bass_kernel_spmd(nc, in_maps, list(range(NCORES)))
    out = np.concatenate([res.results[k]["out_c"] for k in range(NCORES)], axis=1)
    ht = res.results[0]["ht_o"]
    ct = res.results[0]["ct_o"]
    return out, ht, ct


# revision 15
# speedup vs baseline: 1.4171x; 1.4151x over previous
"""DecoderRNN (LSTM cell + Bahdanau attention + vocab projection + log-softmax)
on 8 Trainium2 NeuronCores.

Sharding:
  - Embedding lookup + LSTM cell: replicated on every core (full batch N=64);
    the gate GEMMs are moving-operand-bound, so replication costs no extra PE
    time and avoids gathering h_t before attention.
  - Bahdanau attention: data-parallel over batch (8 rows per core); per-core
    context rows are AllGathered (16 KB).
  - h2o projection: tensor-parallel over vocab (4000 rows per core), split into
    an h_t half (runs while the context AllGather is in flight) and a context
    half; local log-softmax stats (max, sum-exp) are AllGathered (512 B) and
    combined exactly; each core emits its own [64, 4000] output slice.

GEMM operands are bf16 (fp32 PSUM accumulation) - fp32 matmuls on TRN2 lower
to two PE passes, so bf16 halves both PE time and the dominant weight stream.
The log-softmax/stat/pointwise paths stay fp32. Weight matrices and enc/h are
passed pre-transposed (contract dim major) so streaming DMA loads are
contiguous; fp32/bf16 activations have no usable DMA-transpose path here.
"""
import numpy as np

V, E, H, N, L = 32000, 512, 512, 64, 128
NCORES = 8
NB = N // NCORES        # 8 batch rows per core
VS = V // NCORES        # 4000 vocab rows per core
G4 = 4 * H              # 2048 gates
KC = 2 * H              # 1024 contraction dim of h2o
NKC = KC // 128         # 8 k-chunks
NVC = (VS + 511) // 512  # 8 vocab chunks per core (7x512 + 416)

_cached = None


def _build():
    import contextlib
    import concourse.bass as bass
    import concourse.tile as tile
    from concourse import bacc, mybir
    from concourse.masks import make_identity

    f32 = mybir.dt.float32
    bf16 = mybir.dt.bfloat16
    i32 = mybir.dt.int32
    u8 = mybir.dt.uint8
    X = mybir.AxisListType.X
    AF = mybir.ActivationFunctionType
    OP = mybir.AluOpType

    nc = bacc.Bacc("TRN2", target_bir_lowering=False, debug=False,
                   num_devices=NCORES)

    ExtIn = dict(kind="ExternalInput")
    d_ids = nc.dram_tensor("ids", [N, 1], i32, **ExtIn).ap()
    d_hT = nc.dram_tensor("hT", [H, N], f32, **ExtIn).ap()
    d_c = nc.dram_tensor("c0", [N, H], f32, **ExtIn).ap()
    d_encN = nc.dram_tensor("encN", [L, NB, H], bf16, **ExtIn).ap()
    d_encT = nc.dram_tensor("encT", [H, NB * L], bf16, **ExtIn).ap()
    d_msk = nc.dram_tensor("msk", [NB, L], u8, **ExtIn).ap()
    d_embW = nc.dram_tensor("embW", [V, E], f32, **ExtIn).ap()
    d_wihT = nc.dram_tensor("wihT", [E, G4], f32, **ExtIn).ap()
    d_whhT = nc.dram_tensor("whhT", [H, G4], f32, **ExtIn).ap()
    d_bg = nc.dram_tensor("bg", [1, G4], f32, **ExtIn).ap()      # b_ih + b_hh
    d_awhT = nc.dram_tensor("awhT", [H, H], bf16, **ExtIn).ap()  # attn_Wh^T
    d_awsT = nc.dram_tensor("awsT", [H, H], bf16, **ExtIn).ap()  # attn_Ws^T
    d_av = nc.dram_tensor("av", [1, H], bf16, **ExtIn).ap()
    d_howT = nc.dram_tensor("howT", [KC, VS], bf16, **ExtIn).ap()
    d_hob = nc.dram_tensor("hob", [1, VS], f32, **ExtIn).ap()
    d_sel = nc.dram_tensor("sel", [N, NB], f32, **ExtIn).ap()    # one-hot rows

    d_out = nc.dram_tensor("out_c", [N, VS], f32, kind="ExternalOutput").ap()
    d_hto = nc.dram_tensor("ht_o", [N, H], f32, kind="ExternalOutput").ap()
    d_cto = nc.dram_tensor("ct_o", [N, H], f32, kind="ExternalOutput").ap()

    with tile.TileContext(nc) as tc, contextlib.ExitStack() as ctx:
        const = ctx.enter_context(tc.tile_pool(name="const", bufs=1))
        sb = ctx.enter_context(tc.tile_pool(name="sb", bufs=1))
        sb2 = ctx.enter_context(tc.tile_pool(name="sb2", bufs=2))
        lstmw = ctx.enter_context(tc.tile_pool(name="lstmw", bufs=4))
        wstr = ctx.enter_context(tc.tile_pool(name="wstr", bufs=12))
        ppt = ctx.enter_context(tc.tile_pool(name="ppt", bufs=2, space="PSUM"))
        dram = ctx.enter_context(tc.tile_pool(name="dram", bufs=1, space="DRAM"))

        ident = const.tile([128, 128], f32, tag="ident")
        make_identity(nc, ident)
        ones64 = const.tile([1, N], f32, tag="ones64")
        nc.vector.memset(ones64, 1.0)

        def transp(dst_ap, src_ap):
            """PE transpose src [p, f] -> dst [f, p] via PSUM (cast on copy)."""
            p, f = src_ap.shape[0], src_ap.shape[-1]
            tp = ppt.tile([128, 128], f32, tag="tp")
            nc.tensor.transpose(out=tp[:f, :p], in_=src_ap, identity=ident[:p, :p])
            nc.vector.tensor_copy(out=dst_ap, in_=tp[:f, :p])

        # ------------- embedding gather + small input loads -------------
        ids_sb = const.tile([N, 1], i32, tag="ids")
        nc.sync.dma_start(out=ids_sb, in_=d_ids)
        emb_sb = sb.tile([N, E], f32, tag="emb")
        nc.gpsimd.indirect_dma_start(
            out=emb_sb[:, :], out_offset=None, in_=d_embW,
            in_offset=bass.IndirectOffsetOnAxis(ap=ids_sb[:, :1], axis=0))
        c_sb = sb.tile([N, H], f32, tag="c")
        nc.sync.dma_start(out=c_sb, in_=d_c)
        bg_sb = const.tile([1, G4], f32, tag="bg")
        nc.sync.dma_start(out=bg_sb, in_=d_bg)
        hT = sb.tile([128, 4, N], f32, tag="hT")
        nc.sync.dma_start(out=hT, in_=d_hT.rearrange("(kc p) n -> p kc n", p=128))
        encN = sb.tile([L, NB, H], bf16, tag="encN")
        nc.sync.dma_start(out=encN, in_=d_encN)
        encT = sb.tile([128, 4, NB * L], bf16, tag="encT")
        nc.sync.dma_start(out=encT, in_=d_encT.rearrange("(kc p) r -> p kc r", p=128))
        awhT_sb = sb.tile([128, 4, H], bf16, tag="awhT")
        nc.sync.dma_start(out=awhT_sb, in_=d_awhT.rearrange("(kc p) n -> p kc n", p=128))
        awsT_sb = sb.tile([128, 4, H], bf16, tag="awsT")
        nc.sync.dma_start(out=awsT_sb, in_=d_awsT.rearrange("(kc p) n -> p kc n", p=128))
        avT_sb = const.tile([128, 4], bf16, tag="avT")
        nc.sync.dma_start(out=avT_sb, in_=d_av.rearrange("o (k p) -> p (k o)", p=128))
        sel_sb = const.tile([N, NB], f32, tag="sel")
        nc.sync.dma_start(out=sel_sb, in_=d_sel)
        msk_sb = const.tile([NB, L], u8, tag="msk")
        nc.sync.dma_start(out=msk_sb, in_=d_msk)
        hob_sb = sb.tile([1, VS], f32, tag="hob")
        nc.sync.dma_start(out=hob_sb, in_=d_hob)

        # ---------------- LSTM cell (replicated, full batch) ----------------
        embT = sb.tile([128, 4, N], f32, tag="embT")
        for k in range(4):
            transp(embT[:, k, :], emb_sb[:, k * 128:(k + 1) * 128])

        ga = sb.tile([N, 4, H], f32, tag="ga")  # sig_i, sig_f, tanh_g, sig_o
        ph_lstm = contextlib.ExitStack()
        ppg = ph_lstm.enter_context(tc.tile_pool(name="ppg", bufs=2, space="PSUM"))
        for g in range(4):
            gp = ppg.tile([N, H], f32, tag="gp")
            for k in range(4):
                wt = lstmw.tile([128, H], f32, tag="lw")
                nc.sync.dma_start(out=wt, in_=d_wihT[k * 128:(k + 1) * 128,
                                                     g * H:(g + 1) * H])
                nc.tensor.matmul(out=gp, lhsT=embT[:, k, :], rhs=wt,
                                 start=(k == 0), stop=False)
            for k in range(4):
                wt = lstmw.tile([128, H], f32, tag="lw")
                nc.sync.dma_start(out=wt, in_=d_whhT[k * 128:(k + 1) * 128,
                                                     g * H:(g + 1) * H])
                nc.tensor.matmul(out=gp, lhsT=hT[:, k, :], rhs=wt,
                                 start=False, stop=False)
            nc.tensor.matmul(out=gp, lhsT=ones64, rhs=bg_sb[:, g * H:(g + 1) * H],
                             start=False, stop=True)
            nc.scalar.activation(out=ga[:, g, :], in_=gp,
                                 func=AF.Tanh if g == 2 else AF.Sigmoid)

        t1 = sb.tile([N, H], f32, tag="t1")
        nc.vector.tensor_mul(out=t1, in0=ga[:, 1, :], in1=c_sb)        # f*c
        t2 = sb.tile([N, H], f32, tag="t2")
        nc.vector.tensor_mul(out=t2, in0=ga[:, 0, :], in1=ga[:, 2, :])  # i*g
        ct_sb = sb.tile([N, H], f32, tag="ct")
        nc.vector.tensor_add(out=ct_sb, in0=t1, in1=t2)
        tc_sb = sb.tile([N, H], f32, tag="tc")
        nc.scalar.activation(out=tc_sb, in_=ct_sb, func=AF.Tanh)
        ht_sb = sb.tile([N, H], f32, tag="ht")
        nc.vector.tensor_mul(out=ht_sb, in0=ga[:, 3, :], in1=tc_sb)
        nc.sync.dma_start(out=d_cto, in_=ct_sb)
        nc.sync.dma_start(out=d_hto, in_=ht_sb)

        # combT[:, 0:4] = h_t^T ; [:, 4:8] = context^T (after AllGather)
        combT = sb.tile([128, NKC, N], bf16, tag="combT")
        for k in range(4):
            transp(combT[:, k, :], ht_sb[:, k * 128:(k + 1) * 128])

        # ---------------- attention (own NB=8 batch rows) ----------------
        qp = ppg.tile([N, H], f32, tag="qp")
        for k in range(4):
            nc.tensor.matmul(out=qp, lhsT=combT[:, k, :], rhs=awhT_sb[:, k, :],
                             start=(k == 0), stop=(k == 3))
        q_sb = sb.tile([N, H], f32, tag="q")
        nc.vector.tensor_copy(out=q_sb, in_=qp)
        qop = ppg.tile([NB, H], f32, tag="qop")
        nc.tensor.matmul(out=qop, lhsT=sel_sb, rhs=q_sb, start=True, stop=True)
        qo_sb = sb.tile([NB, H], f32, tag="qo")
        nc.vector.tensor_copy(out=qo_sb, in_=qop)
        ph_lstm.close()
        ph_attn = contextlib.ExitStack()
        ppa = ph_attn.enter_context(tc.tile_pool(name="ppa", bufs=2, space="PSUM"))
        pps = ph_attn.enter_context(tc.tile_pool(name="pps", bufs=2, space="PSUM"))
        # q_own^T: [128 h, 4 kc, 8 b] fp32 bias columns for the fused tanh
        qoT = sb.tile([128, 4, NB], f32, tag="qoT")
        for k in range(4):
            transp(qoT[:, k, :], qo_sb[:, k * 128:(k + 1) * 128])

        # proj^T = Ws @ enc^T (+ per-batch q bias inside tanh), then v-dot.
        sstage = sb.tile([1, NB, L], f32, tag="sstage")
        for nch in range(2):
            scp = pps.tile([1, 4 * L], f32, tag="scp")
            for mh in range(4):
                pp = ppa.tile([128, 4 * L], f32, tag="pp")
                for k in range(4):
                    nc.tensor.matmul(
                        out=pp,
                        lhsT=awsT_sb[:, k, mh * 128:(mh + 1) * 128],
                        rhs=encT[:, k, nch * 512:(nch + 1) * 512],
                        start=(k == 0), stop=(k == 3))
                th = sb2.tile([128, 4 * L], bf16, tag="th")
                for b in range(4):
                    j = nch * 4 + b
                    nc.scalar.activation(out=th[:, b * L:(b + 1) * L],
                                         in_=pp[:, b * L:(b + 1) * L],
                                         func=AF.Tanh, bias=qoT[:, mh, j:j + 1])
                nc.tensor.matmul(out=scp, lhsT=avT_sb[:, mh:mh + 1], rhs=th,
                                 start=(mh == 0), stop=(mh == 3))
            nc.vector.tensor_copy(out=sstage[0:1, nch * 4:(nch + 1) * 4, :],
                                  in_=scp[0:1, :].rearrange("p (b l) -> p b l", l=L))
        sc_sb = sb.tile([NB, L], f32, tag="sc")
        nc.sync.dma_start(out=sc_sb, in_=sstage)  # [1,8,128] -> [8,128] scatter

        # mask + softmax over l (fp32)
        mskf = sb.tile([NB, L], f32, tag="mskf")
        nc.vector.tensor_copy(out=mskf, in_=msk_sb)
        nc.scalar.mul(mskf, mskf, -1e30)
        nc.vector.tensor_add(out=sc_sb, in0=sc_sb, in1=mskf)
        amx = sb.tile([NB, 1], f32, tag="amx")
        nc.vector.reduce_max(out=amx, in_=sc_sb, axis=X)
        namx = sb.tile([NB, 1], f32, tag="namx")
        nc.scalar.mul(namx, amx, -1.0)
        aew = sb.tile([NB, L], f32, tag="aew")
        asw = sb.tile([NB, 1], f32, tag="asw")
        nc.scalar.activation(out=aew, in_=sc_sb, func=AF.Exp, bias=namx,
                             accum_out=asw)
        arw = sb.tile([NB, 1], f32, tag="arw")
        nc.vector.reciprocal(out=arw, in_=asw)
        aw_sb = sb.tile([NB, L], f32, tag="aw")
        nc.vector.tensor_scalar_mul(aw_sb, aew, arw)

        # attn_w^T [128 l, 8 b] (bf16), context rows via M=1 matmuls
        awt = sb.tile([L, NB], bf16, tag="awt")
        transp(awt, aw_sb)
        cstage = sb.tile([1, NB, H], f32, tag="cstage")
        for j in range(NB):
            cp = pps.tile([1, H], f32, tag="cp")
            nc.tensor.matmul(out=cp, lhsT=awt[:, j:j + 1], rhs=encN[:, j, :],
                             start=True, stop=True)
            nc.vector.tensor_copy(out=cstage[0:1, j, :], in_=cp[0:1, :])
        ph_attn.close()
        ppm = ctx.enter_context(tc.tile_pool(name="ppm", bufs=4, space="PSUM"))

        # ---------------- h2o projection phase A: h_t half ----------------
        # Runs while the context AllGather is in flight.
        logits = sb.tile([N, VS], f32, tag="logits")
        for npair in range(4):
            w_a = []
            cw = min(1024, VS - npair * 1024)
            for k in range(4):
                wt = wstr.tile([128, 1024], bf16, tag="wt")
                nc.sync.dma_start(out=wt[:, :cw],
                                  in_=d_howT[k * 128:(k + 1) * 128,
                                             npair * 1024:npair * 1024 + cw])
                w_a.append(wt)
            for half in range(2):
                n = npair * 2 + half
                cs = min(512, VS - n * 512)
                nsl = slice(n * 512, n * 512 + cs)
                pmt = ppm.tile([N, 512], f32, tag="pmt")
                for k in range(4):
                    nc.tensor.matmul(out=pmt[:, :cs], lhsT=combT[:, k, :],
                                     rhs=w_a[k][:, half * 512:half * 512 + cs],
                                     start=(k == 0), stop=False)
                nc.tensor.matmul(out=pmt[:, :cs], lhsT=ones64,
                                 rhs=hob_sb[:, nsl], start=False, stop=True)
                nc.vector.tensor_copy(out=logits[:, nsl], in_=pmt[:, :cs])

        # ---------------- AllGather context ----------------
        cb_in = dram.tile([NB, H], f32, tag="cbi")
        nc.sync.dma_start(out=cb_in, in_=cstage)
        cb_out = dram.tile([N, H], f32, tag="cbo")
        nc.gpsimd.collective_compute(
            "AllGather", OP.bypass, replica_groups=[list(range(NCORES))],
            ins=[cb_in[:, :].opt()], outs=[cb_out[:, :].opt()])
        ctxg = sb.tile([N, H], f32, tag="ctxg")
        nc.sync.dma_start(out=ctxg, in_=cb_out)
        for k in range(4):
            transp(combT[:, 4 + k, :], ctxg[:, k * 128:(k + 1) * 128])

        # ---------------- phase B: context half + per-chunk stats ----------
        rmx8 = sb.tile([N, NVC], f32, tag="rmx8")
        nrmx8 = sb.tile([N, NVC], f32, tag="nrmx8")
        sacc = sb.tile([N, NVC], f32, tag="sacc")
        for npair in range(4):
            w_b = []
            cw = min(1024, VS - npair * 1024)
            for k in range(4):
                wt = wstr.tile([128, 1024], bf16, tag="wt")
                nc.sync.dma_start(out=wt[:, :cw],
                                  in_=d_howT[(4 + k) * 128:(5 + k) * 128,
                                             npair * 1024:npair * 1024 + cw])
                w_b.append(wt)
            for half in range(2):
                n = npair * 2 + half
                cs = min(512, VS - n * 512)
                nsl = slice(n * 512, n * 512 + cs)
                pmt = ppm.tile([N, 512], f32, tag="pmt")
                for k in range(4):
                    nc.tensor.matmul(out=pmt[:, :cs], lhsT=combT[:, 4 + k, :],
                                     rhs=w_b[k][:, half * 512:half * 512 + cs],
                                     start=(k == 0), stop=(k == 3))
                nc.vector.tensor_add(out=logits[:, nsl], in0=logits[:, nsl],
                                     in1=pmt[:, :cs])
                nc.vector.reduce_max(out=rmx8[:, n:n + 1], in_=logits[:, nsl],
                                     axis=X)
                nc.vector.tensor_scalar_mul(nrmx8[:, n:n + 1], rmx8[:, n:n + 1],
                                            -1.0)
                junk = sb2.tile([N, 512], f32, tag="junk")
                nc.scalar.activation(out=junk[:, :cs], in_=logits[:, nsl],
                                     func=AF.Exp, bias=nrmx8[:, n:n + 1],
                                     accum_out=sacc[:, n:n + 1])

        # combine per-chunk stats: m = max_n m_n; S = sum_n s_n*e^(m_n-m)
        mloc = sb.tile([N, 1], f32, tag="mloc")
        nc.vector.reduce_max(out=mloc, in_=rmx8, axis=X)
        dmn = sb.tile([N, NVC], f32, tag="dmn")
        nc.vector.tensor_tensor(out=dmn, in0=rmx8,
                                in1=mloc.to_broadcast([N, NVC]), op=OP.subtract)
        edmn = sb.tile([N, NVC], f32, tag="edmn")
        nc.scalar.activation(out=edmn, in_=dmn, func=AF.Exp)
        wsn = sb.tile([N, NVC], f32, tag="wsn")
        nc.vector.tensor_mul(out=wsn, in0=edmn, in1=sacc)
        sloc = sb.tile([N, 1], f32, tag="sloc")
        nc.vector.reduce_sum(out=sloc, in_=wsn, axis=X)

        # ---------------- AllGather (max, sumexp) stats ----------------
        st_sb = sb.tile([N, 2], f32, tag="st")
        nc.vector.tensor_copy(out=st_sb[:, 0:1], in_=mloc)
        nc.vector.tensor_copy(out=st_sb[:, 1:2], in_=sloc)
        stb_in = dram.tile([N, 2], f32, tag="sbi")
        nc.sync.dma_start(out=stb_in, in_=st_sb)
        stb_out = dram.tile([NCORES * N, 2], f32, tag="sbo")
        nc.gpsimd.collective_compute(
            "AllGather", OP.bypass, replica_groups=[list(range(NCORES))],
            ins=[stb_in[:, :].opt()], outs=[stb_out[:, :].opt()])
        sall = sb.tile([N, NCORES, 2], f32, tag="sall")
        nc.sync.dma_start(out=sall,
                          in_=stb_out[:, :].rearrange("(r n) s -> n r s", n=N))
        mg = sb.tile([N, 1], f32, tag="mg")
        nc.vector.reduce_max(out=mg, in_=sall[:, :, 0], axis=X)
        dmm = sb.tile([N, NCORES], f32, tag="dmm")
        nc.vector.tensor_tensor(out=dmm, in0=sall[:, :, 0],
                                in1=mg.to_broadcast([N, NCORES]), op=OP.subtract)
        edm = sb.tile([N, NCORES], f32, tag="edm")
        nc.scalar.activation(out=edm, in_=dmm, func=AF.Exp)
        wse = sb.tile([N, NCORES], f32, tag="wse")
        nc.vector.tensor_mul(out=wse, in0=edm, in1=sall[:, :, 1])
        sg = sb.tile([N, 1], f32, tag="sg")
        nc.vector.reduce_sum(out=sg, in_=wse, axis=X)
        lsg = sb.tile([N, 1], f32, tag="lsg")
        nc.scalar.activation(out=lsg, in_=sg, func=AF.Ln)
        off = sb.tile([N, 1], f32, tag="off")
        nc.vector.tensor_add(out=off, in0=mg, in1=lsg)
        noff = sb.tile([N, 1], f32, tag="noff")
        nc.vector.tensor_scalar_mul(noff, off, -1.0)

        # final: out = logits - (Mg + log Sg), chunked so ACT overlaps DMA out
        for n in range(NVC):
            cs = min(512, VS - n * 512)
            nsl = slice(n * 512, n * 512 + cs)
            foc = sb2.tile([N, 512], f32, tag="foc")
            nc.scalar.activation(out=foc[:, :cs], in_=logits[:, nsl],
                                 func=AF.Identity, bias=noff, scale=1.0)
            nc.sync.dma_start(out=d_out[:, nsl], in_=foc[:, :cs])

    nc.compile()
    return nc


def _get_nc():
    global _cached
    if _cached is None:
        _cached = _build()
    return _cached


def _make_in_maps(input_ids, h, c, encoder_hiddens, attn_mask, embed_W,
                  w_ih, b_ih, w_hh, b_hh, attn_Wh, attn_Ws, attn_v,
                  h2o_W, h2o_b):
    import ml_dtypes
    bf = ml_dtypes.bfloat16
    f = lambda a: np.asarray(a, dtype=np.float32)

    ids32 = np.asarray(input_ids).astype(np.int32).reshape(N, 1)
    hT = np.ascontiguousarray(f(h).T)                       # [H, N]
    c = np.ascontiguousarray(f(c))
    enc = f(encoder_hiddens)                                # [N, L, H]
    msk = np.asarray(attn_mask).astype(np.uint8)
    embW = np.ascontiguousarray(f(embed_W))
    wihT = np.ascontiguousarray(f(w_ih).T)                  # [E, 4H]
    whhT = np.ascontiguousarray(f(w_hh).T)                  # [H, 4H]
    bg = (np.asarray(b_ih, np.float64) + np.asarray(b_hh, np.float64)) \
        .astype(np.float32).reshape(1, G4)
    awhT = np.ascontiguousarray(f(attn_Wh).T.astype(bf))
    awsT = np.ascontiguousarray(f(attn_Ws).T.astype(bf))
    av = f(attn_v).reshape(1, H).astype(bf)
    hoWT = f(h2o_W).T.astype(bf)                            # [2H, V]
    hob = f(h2o_b)

    in_maps = []
    for k in range(NCORES):
        sel = np.zeros((N, NB), np.float32)
        sel[np.arange(k * NB, (k + 1) * NB), np.arange(NB)] = 1.0
        encs = enc[k * NB:(k + 1) * NB]                     # [NB, L, H]
        in_maps.append({
            "ids": ids32,
            "hT": hT,
            "c0": c,
            "encN": np.ascontiguousarray(
                encs.transpose(1, 0, 2).astype(bf)),        # [L, NB, H]
            "encT": np.ascontiguousarray(
                encs.reshape(NB * L, H).T.astype(bf)),      # [H, NB*L]
            "msk": np.ascontiguousarray(msk[k * NB:(k + 1) * NB]),
            "embW": embW,
            "wihT": wihT,
            "whhT": whhT,
            "bg": bg,
            "awhT": awhT,
            "awsT": awsT,
            "av": av,
            "howT": np.ascontiguousarray(hoWT[:, k * VS:(k + 1) * VS]),
            "hob": np.ascontiguousarray(hob[k * VS:(k + 1) * VS]).reshape(1, VS),
            "sel": sel,
        })
    return in_maps


def kernel(**inputs):
    from concourse.bass_utils import run_bass_kernel_spmd

    nc = _get_nc()
    in_maps = _make_in_maps(**inputs)
    res = run_bass_kernel_spmd(nc, in_maps, list(range(NCORES)))
    out = np.concatenate([res.results[k]["out_c"] for k in range(NCORES)], axis=1)
    ht = res.results[0]["ht_o"]
    ct = res.results[0]["ct_o"]
    return out, ht, ct


# revision 16
# speedup vs baseline: 1.6568x; 1.1691x over previous
"""DecoderRNN (LSTM cell + Bahdanau attention + vocab projection + log-softmax)
on 8 Trainium2 NeuronCores.

Sharding:
  - Embedding lookup + LSTM cell: replicated on every core (full batch N=64);
    the gate GEMMs are moving-operand-bound, so replication costs no extra PE
    time and avoids gathering h_t before attention.
  - Bahdanau attention: data-parallel over batch (8 rows per core); per-core
    context rows are AllGathered (16 KB).
  - h2o projection: tensor-parallel over vocab (4000 rows per core), split into
    an h_t half (runs while the context AllGather is in flight) and a context
    half; local log-softmax stats (max, sum-exp) are AllGathered (512 B) and
    combined exactly; each core emits its own [64, 4000] output slice.

GEMM operands are bf16 (fp32 PSUM accumulation) - fp32 matmuls on TRN2 lower
to two PE passes, so bf16 halves both PE time and the dominant weight stream.
The log-softmax/stat/pointwise paths stay fp32. Weight matrices and enc/h are
passed pre-transposed (contract dim major) so streaming DMA loads are
contiguous; fp32/bf16 activations have no usable DMA-transpose path here.
"""
import numpy as np

V, E, H, N, L = 32000, 512, 512, 64, 128
NCORES = 8
NB = N // NCORES        # 8 batch rows per core
VS = V // NCORES        # 4000 vocab rows per core
G4 = 4 * H              # 2048 gates
KC = 2 * H              # 1024 contraction dim of h2o
NKC = KC // 128         # 8 k-chunks
NVC = (VS + 511) // 512  # 8 vocab chunks per core (7x512 + 416)

_cached = None


def _build():
    import contextlib
    import concourse.bass as bass
    import concourse.tile as tile
    from concourse import bacc, mybir
    from concourse.masks import make_identity

    f32 = mybir.dt.float32
    bf16 = mybir.dt.bfloat16
    i32 = mybir.dt.int32
    u8 = mybir.dt.uint8
    X = mybir.AxisListType.X
    AF = mybir.ActivationFunctionType
    OP = mybir.AluOpType

    nc = bacc.Bacc("TRN2", target_bir_lowering=False, debug=False,
                   num_devices=NCORES)

    ExtIn = dict(kind="ExternalInput")
    d_ids = nc.dram_tensor("ids", [N, 1], i32, **ExtIn).ap()
    d_hT = nc.dram_tensor("hT", [H, N], f32, **ExtIn).ap()
    d_c = nc.dram_tensor("c0", [N, H], f32, **ExtIn).ap()
    d_encN = nc.dram_tensor("encN", [L, NB, H], bf16, **ExtIn).ap()
    d_encT = nc.dram_tensor("encT", [H, NB * L], bf16, **ExtIn).ap()
    d_msk = nc.dram_tensor("msk", [NB, L], u8, **ExtIn).ap()
    d_embW = nc.dram_tensor("embW", [V, E], f32, **ExtIn).ap()
    d_wihT = nc.dram_tensor("wihT", [E, G4], f32, **ExtIn).ap()
    d_whhT = nc.dram_tensor("whhT", [H, G4], f32, **ExtIn).ap()
    d_bg = nc.dram_tensor("bg", [1, G4], f32, **ExtIn).ap()      # b_ih + b_hh
    d_awhT = nc.dram_tensor("awhT", [H, H], bf16, **ExtIn).ap()  # attn_Wh^T
    d_awsT = nc.dram_tensor("awsT", [H, H], bf16, **ExtIn).ap()  # attn_Ws^T
    d_av = nc.dram_tensor("av", [1, H], bf16, **ExtIn).ap()
    d_howT = nc.dram_tensor("howT", [KC, VS], bf16, **ExtIn).ap()
    d_hob = nc.dram_tensor("hob", [1, VS], f32, **ExtIn).ap()
    d_sel = nc.dram_tensor("sel", [N, NB], f32, **ExtIn).ap()    # one-hot rows

    d_out = nc.dram_tensor("out_c", [N, VS], f32, kind="ExternalOutput").ap()
    d_hto = nc.dram_tensor("ht_o", [N, H], f32, kind="ExternalOutput").ap()
    d_cto = nc.dram_tensor("ct_o", [N, H], f32, kind="ExternalOutput").ap()

    with tile.TileContext(nc) as tc, contextlib.ExitStack() as ctx:
        const = ctx.enter_context(tc.tile_pool(name="const", bufs=1))
        sb = ctx.enter_context(tc.tile_pool(name="sb", bufs=1))
        sb2 = ctx.enter_context(tc.tile_pool(name="sb2", bufs=2))
        lstmw = ctx.enter_context(tc.tile_pool(name="lstmw", bufs=8))
        wstr = ctx.enter_context(tc.tile_pool(name="wstr", bufs=16))
        ppt = ctx.enter_context(tc.tile_pool(name="ppt", bufs=2, space="PSUM"))
        dram = ctx.enter_context(tc.tile_pool(name="dram", bufs=1, space="DRAM"))

        ident = const.tile([128, 128], f32, tag="ident")
        make_identity(nc, ident)
        ones64 = const.tile([1, N], f32, tag="ones64")
        nc.vector.memset(ones64, 1.0)

        def transp(dst_ap, src_ap):
            """PE transpose src [p, f] -> dst [f, p] via PSUM (cast on copy)."""
            p, f = src_ap.shape[0], src_ap.shape[-1]
            tp = ppt.tile([128, 128], f32, tag="tp")
            nc.tensor.transpose(out=tp[:f, :p], in_=src_ap, identity=ident[:p, :p])
            nc.vector.tensor_copy(out=dst_ap, in_=tp[:f, :p])

        # ------------- early loads: attention operands first -------------
        encT = sb.tile([128, 4, NB * L], bf16, tag="encT")
        nc.sync.dma_start(out=encT, in_=d_encT.rearrange("(kc p) r -> p kc r", p=128))
        awsT_sb = sb.tile([128, 4, H], bf16, tag="awsT")
        nc.sync.dma_start(out=awsT_sb, in_=d_awsT.rearrange("(kc p) n -> p kc n", p=128))

        # ------------- embedding gather + small input loads -------------
        ids_sb = const.tile([N, 1], i32, tag="ids")
        nc.sync.dma_start(out=ids_sb, in_=d_ids)
        emb_sb = sb.tile([N, E], f32, tag="emb")
        nc.gpsimd.indirect_dma_start(
            out=emb_sb[:, :], out_offset=None, in_=d_embW,
            in_offset=bass.IndirectOffsetOnAxis(ap=ids_sb[:, :1], axis=0))
        c_sb = sb.tile([N, H], f32, tag="c")
        nc.sync.dma_start(out=c_sb, in_=d_c)
        bg_sb = const.tile([1, G4], f32, tag="bg")
        nc.sync.dma_start(out=bg_sb, in_=d_bg)
        hT = sb.tile([128, 4, N], f32, tag="hT")
        nc.sync.dma_start(out=hT, in_=d_hT.rearrange("(kc p) n -> p kc n", p=128))
        encN = sb.tile([L, NB, H], bf16, tag="encN")
        nc.sync.dma_start(out=encN, in_=d_encN)
        awhT_sb = sb.tile([128, 4, H], bf16, tag="awhT")
        nc.sync.dma_start(out=awhT_sb, in_=d_awhT.rearrange("(kc p) n -> p kc n", p=128))
        avT_sb = const.tile([128, 4], bf16, tag="avT")
        nc.sync.dma_start(out=avT_sb, in_=d_av.rearrange("o (k p) -> p (k o)", p=128))
        sel_sb = const.tile([N, NB], f32, tag="sel")
        nc.sync.dma_start(out=sel_sb, in_=d_sel)
        msk_sb = const.tile([NB, L], u8, tag="msk")
        nc.sync.dma_start(out=msk_sb, in_=d_msk)
        hob_sb = sb.tile([1, VS], f32, tag="hob")
        nc.sync.dma_start(out=hob_sb, in_=d_hob)

        # ---------------- LSTM cell (replicated, full batch) ----------------
        embT = sb.tile([128, 4, N], f32, tag="embT")
        for k in range(4):
            transp(embT[:, k, :], emb_sb[:, k * 128:(k + 1) * 128])

        ga = sb.tile([N, 4, H], f32, tag="ga")  # sig_i, sig_f, tanh_g, sig_o
        ph_lstm = contextlib.ExitStack()
        ppg = ph_lstm.enter_context(tc.tile_pool(name="ppg", bufs=2, space="PSUM"))
        ppa = ph_lstm.enter_context(tc.tile_pool(name="ppa", bufs=2, space="PSUM"))
        pps = ph_lstm.enter_context(tc.tile_pool(name="pps", bufs=2, space="PSUM"))
        for g in range(4):
            gp = ppg.tile([N, H], f32, tag="gp")
            for k in range(4):
                wt = lstmw.tile([128, H], f32, tag="lw")
                nc.sync.dma_start(out=wt, in_=d_wihT[k * 128:(k + 1) * 128,
                                                     g * H:(g + 1) * H])
                nc.tensor.matmul(out=gp, lhsT=embT[:, k, :], rhs=wt,
                                 start=(k == 0), stop=False)
            for k in range(4):
                wt = lstmw.tile([128, H], f32, tag="lw")
                nc.sync.dma_start(out=wt, in_=d_whhT[k * 128:(k + 1) * 128,
                                                     g * H:(g + 1) * H])
                nc.tensor.matmul(out=gp, lhsT=hT[:, k, :], rhs=wt,
                                 start=False, stop=False)
            nc.tensor.matmul(out=gp, lhsT=ones64, rhs=bg_sb[:, g * H:(g + 1) * H],
                             start=False, stop=True)
            nc.scalar.activation(out=ga[:, g, :], in_=gp,
                                 func=AF.Tanh if g == 2 else AF.Sigmoid)

        t1 = sb.tile([N, H], f32, tag="t1")
        nc.vector.tensor_mul(out=t1, in0=ga[:, 1, :], in1=c_sb)        # f*c
        t2 = sb.tile([N, H], f32, tag="t2")
        nc.vector.tensor_mul(out=t2, in0=ga[:, 0, :], in1=ga[:, 2, :])  # i*g
        ct_sb = sb.tile([N, H], f32, tag="ct")
        nc.vector.tensor_add(out=ct_sb, in0=t1, in1=t2)
        tc_sb = sb.tile([N, H], f32, tag="tc")
        nc.scalar.activation(out=tc_sb, in_=ct_sb, func=AF.Tanh)
        ht_sb = sb.tile([N, H], f32, tag="ht")
        nc.vector.tensor_mul(out=ht_sb, in0=ga[:, 3, :], in1=tc_sb)
        nc.sync.dma_start(out=d_cto, in_=ct_sb)
        nc.sync.dma_start(out=d_hto, in_=ht_sb)

        # combT[:, 0:4] = h_t^T ; [:, 4:8] = context^T (after AllGather)
        combT = sb.tile([128, NKC, N], bf16, tag="combT")
        for k in range(4):
            transp(combT[:, k, :], ht_sb[:, k * 128:(k + 1) * 128])

        # ---------------- attention (own NB=8 batch rows) ----------------
        qp = ppg.tile([N, H], f32, tag="gp")
        for k in range(4):
            nc.tensor.matmul(out=qp, lhsT=combT[:, k, :], rhs=awhT_sb[:, k, :],
                             start=(k == 0), stop=(k == 3))
        q_sb = sb.tile([N, H], f32, tag="q")
        nc.vector.tensor_copy(out=q_sb, in_=qp)
        qop = ppg.tile([NB, H], f32, tag="gp")
        nc.tensor.matmul(out=qop, lhsT=sel_sb, rhs=q_sb, start=True, stop=True)
        qo_sb = sb.tile([NB, H], f32, tag="qo")
        nc.vector.tensor_copy(out=qo_sb, in_=qop)
        # q_own^T: [128 h, 4 kc, 8 b] fp32 bias columns for the fused tanh
        qoT = sb.tile([128, 4, NB], f32, tag="qoT")
        for k in range(4):
            transp(qoT[:, k, :], qo_sb[:, k * 128:(k + 1) * 128])

        # proj^T = Ws @ enc^T (+ per-batch q bias inside tanh), then v-dot.
        sstage = sb.tile([1, NB, L], f32, tag="sstage")
        for nch in range(2):
            scp = pps.tile([1, 4 * L], f32, tag="sc")
            for mh in range(4):
                pp = ppa.tile([128, 4 * L], f32, tag="pp")
                for k in range(4):
                    nc.tensor.matmul(
                        out=pp,
                        lhsT=awsT_sb[:, k, mh * 128:(mh + 1) * 128],
                        rhs=encT[:, k, nch * 512:(nch + 1) * 512],
                        start=(k == 0), stop=(k == 3))
                th = sb2.tile([128, 4 * L], bf16, tag="th")
                for b in range(4):
                    j = nch * 4 + b
                    nc.scalar.activation(out=th[:, b * L:(b + 1) * L],
                                         in_=pp[:, b * L:(b + 1) * L],
                                         func=AF.Tanh, bias=qoT[:, mh, j:j + 1])
                nc.tensor.matmul(out=scp, lhsT=avT_sb[:, mh:mh + 1], rhs=th,
                                 start=(mh == 0), stop=(mh == 3))
            nc.vector.tensor_copy(out=sstage[0:1, nch * 4:(nch + 1) * 4, :],
                                  in_=scp[0:1, :].rearrange("p (b l) -> p b l", l=L))
        sc_sb = sb.tile([NB, L], f32, tag="sc")
        nc.sync.dma_start(out=sc_sb, in_=sstage)  # [1,8,128] -> [8,128] scatter

        # mask + softmax over l (fp32)
        mskf = sb.tile([NB, L], f32, tag="mskf")
        nc.vector.tensor_scalar_mul(mskf, msk_sb, -1e30)
        nc.vector.tensor_add(out=sc_sb, in0=sc_sb, in1=mskf)
        namx = sb.tile([NB, 1], f32, tag="namx")
        nc.vector.reduce_max(out=namx, in_=sc_sb, axis=X, negate=True)
        aew = sb.tile([NB, L], f32, tag="aew")
        asw = sb.tile([NB, 1], f32, tag="asw")
        nc.scalar.activation(out=aew, in_=sc_sb, func=AF.Exp, bias=namx,
                             accum_out=asw)
        arw = sb.tile([NB, 1], f32, tag="arw")
        nc.vector.reciprocal(out=arw, in_=asw)
        aw_sb = sb.tile([NB, L], f32, tag="aw")
        nc.vector.tensor_scalar_mul(aw_sb, aew, arw)

        # attn_w^T [128 l, 8 b] (bf16), context rows via M=1 matmuls
        awt = sb.tile([L, NB], bf16, tag="awt")
        transp(awt, aw_sb)
        cstage = sb.tile([1, NB, H], f32, tag="cstage")
        for j in range(NB):
            cp = pps.tile([1, H], f32, tag="sc")
            nc.tensor.matmul(out=cp, lhsT=awt[:, j:j + 1], rhs=encN[:, j, :],
                             start=True, stop=True)
            nc.vector.tensor_copy(out=cstage[0:1, j, :], in_=cp[0:1, :])
        ph_lstm.close()
        ppm = ctx.enter_context(tc.tile_pool(name="ppm", bufs=4, space="PSUM"))

        # ---------------- h2o projection phase A: h_t half ----------------
        # Runs while the context AllGather is in flight.
        logits = sb.tile([N, VS], f32, tag="logits")
        for npair in range(4):
            w_a = []
            cw = min(1024, VS - npair * 1024)
            for k in range(4):
                wt = wstr.tile([128, 1024], bf16, tag="wt")
                nc.sync.dma_start(out=wt[:, :cw],
                                  in_=d_howT[k * 128:(k + 1) * 128,
                                             npair * 1024:npair * 1024 + cw])
                w_a.append(wt)
            for half in range(2):
                n = npair * 2 + half
                cs = min(512, VS - n * 512)
                nsl = slice(n * 512, n * 512 + cs)
                pmt = ppm.tile([N, 512], f32, tag="pmt")
                for k in range(4):
                    nc.tensor.matmul(out=pmt[:, :cs], lhsT=combT[:, k, :],
                                     rhs=w_a[k][:, half * 512:half * 512 + cs],
                                     start=(k == 0), stop=False)
                nc.tensor.matmul(out=pmt[:, :cs], lhsT=ones64,
                                 rhs=hob_sb[:, nsl], start=False, stop=True)
                nc.vector.tensor_copy(out=logits[:, nsl], in_=pmt[:, :cs])

        # ---------------- AllGather context ----------------
        cb_in = dram.tile([NB, H], f32, tag="cbi")
        nc.sync.dma_start(out=cb_in, in_=cstage)
        cb_out = dram.tile([N, H], f32, tag="cbo")
        nc.gpsimd.collective_compute(
            "AllGather", OP.bypass, replica_groups=[list(range(NCORES))],
            ins=[cb_in[:, :].opt()], outs=[cb_out[:, :].opt()])
        ctxg = sb.tile([N, H], f32, tag="ctxg")
        nc.sync.dma_start(out=ctxg, in_=cb_out)
        for k in range(4):
            transp(combT[:, 4 + k, :], ctxg[:, k * 128:(k + 1) * 128])

        # ---------------- phase B: context half + per-chunk stats ----------
        rmx8 = sb.tile([N, NVC], f32, tag="rmx8")
        nrmx8 = sb.tile([N, NVC], f32, tag="nrmx8")
        sacc = sb.tile([N, NVC], f32, tag="sacc")
        for npair in range(4):
            w_b = []
            cw = min(1024, VS - npair * 1024)
            for k in range(4):
                wt = wstr.tile([128, 1024], bf16, tag="wt")
                nc.sync.dma_start(out=wt[:, :cw],
                                  in_=d_howT[(4 + k) * 128:(5 + k) * 128,
                                             npair * 1024:npair * 1024 + cw])
                w_b.append(wt)
            for half in range(2):
                n = npair * 2 + half
                cs = min(512, VS - n * 512)
                nsl = slice(n * 512, n * 512 + cs)
                pmt = ppm.tile([N, 512], f32, tag="pmt")
                for k in range(4):
                    nc.tensor.matmul(out=pmt[:, :cs], lhsT=combT[:, 4 + k, :],
                                     rhs=w_b[k][:, half * 512:half * 512 + cs],
                                     start=(k == 0), stop=(k == 3))
                nc.vector.tensor_add(out=logits[:, nsl], in0=logits[:, nsl],
                                     in1=pmt[:, :cs])
                nc.vector.reduce_max(out=nrmx8[:, n:n + 1], in_=logits[:, nsl],
                                     axis=X, negate=True)
                junk = sb2.tile([N, 512], f32, tag="junk")
                nc.scalar.activation(out=junk[:, :cs], in_=logits[:, nsl],
                                     func=AF.Exp, bias=nrmx8[:, n:n + 1],
                                     accum_out=sacc[:, n:n + 1])

        # combine per-chunk stats: m = max_n m_n; S = sum_n s_n*e^(m_n-m)
        nc.vector.tensor_scalar_mul(rmx8, nrmx8, -1.0)
        mloc = sb.tile([N, 1], f32, tag="mloc")
        nc.vector.reduce_max(out=mloc, in_=rmx8, axis=X)
        dmn = sb.tile([N, NVC], f32, tag="dmn")
        nc.vector.tensor_tensor(out=dmn, in0=rmx8,
                                in1=mloc.to_broadcast([N, NVC]), op=OP.subtract)
        edmn = sb.tile([N, NVC], f32, tag="edmn")
        nc.scalar.activation(out=edmn, in_=dmn, func=AF.Exp)
        wsn = sb.tile([N, NVC], f32, tag="wsn")
        nc.vector.tensor_mul(out=wsn, in0=edmn, in1=sacc)
        sloc = sb.tile([N, 1], f32, tag="sloc")
        nc.vector.reduce_sum(out=sloc, in_=wsn, axis=X)

        # ---------------- AllGather (max, sumexp) stats ----------------
        st_sb = sb.tile([N, 2], f32, tag="st")
        nc.vector.tensor_copy(out=st_sb[:, 0:1], in_=mloc)
        nc.vector.tensor_copy(out=st_sb[:, 1:2], in_=sloc)
        stb_in = dram.tile([N, 2], f32, tag="sbi")
        nc.sync.dma_start(out=stb_in, in_=st_sb)
        stb_out = dram.tile([NCORES * N, 2], f32, tag="sbo")
        nc.gpsimd.collective_compute(
            "AllGather", OP.bypass, replica_groups=[list(range(NCORES))],
            ins=[stb_in[:, :].opt()], outs=[stb_out[:, :].opt()])
        sall = sb.tile([N, NCORES, 2], f32, tag="sall")
        nc.sync.dma_start(out=sall,
                          in_=stb_out[:, :].rearrange("(r n) s -> n r s", n=N))
        mg = sb.tile([N, 1], f32, tag="mg")
        nc.vector.reduce_max(out=mg, in_=sall[:, :, 0], axis=X)
        dmm = sb.tile([N, NCORES], f32, tag="dmm")
        nc.vector.tensor_tensor(out=dmm, in0=sall[:, :, 0],
                                in1=mg.to_broadcast([N, NCORES]), op=OP.subtract)
        edm = sb.tile([N, NCORES], f32, tag="edm")
        nc.scalar.activation(out=edm, in_=dmm, func=AF.Exp)
        wse = sb.tile([N, NCORES], f32, tag="wse")
        nc.vector.tensor_mul(out=wse, in0=edm, in1=sall[:, :, 1])
        sg = sb.tile([N, 1], f32, tag="sg")
        nc.vector.reduce_sum(out=sg, in_=wse, axis=X)
        lsg = sb.tile([N, 1], f32, tag="lsg")
        nc.scalar.activation(out=lsg, in_=sg, func=AF.Ln)
        off = sb.tile([N, 1], f32, tag="off")
        nc.vector.tensor_add(out=off, in0=mg, in1=lsg)
        noff = sb.tile([N, 1], f32, tag="noff")
        nc.vector.tensor_scalar_mul(noff, off, -1.0)

        # final: out = logits - (Mg + log Sg), chunked so ACT overlaps DMA out
        for n in range(2):
            cs = VS // 2
            nsl = slice(n * cs, (n + 1) * cs)
            foc = sb2.tile([N, VS // 2], f32, tag="foc")
            nc.scalar.activation(out=foc, in_=logits[:, nsl],
                                 func=AF.Identity, bias=noff, scale=1.0)
            nc.sync.dma_start(out=d_out[:, nsl], in_=foc)

    nc.compile()
    return nc


def _get_nc():
    global _cached
    if _cached is None:
        _cached = _build()
    return _cached


def _make_in_maps(input_ids, h, c, encoder_hiddens, attn_mask, embed_W,
                  w_ih, b_ih, w_hh, b_hh, attn_Wh, attn_Ws, attn_v,
                  h2o_W, h2o_b):
    import ml_dtypes
    bf = ml_dtypes.bfloat16
    f = lambda a: np.asarray(a, dtype=np.float32)

    ids32 = np.asarray(input_ids).astype(np.int32).reshape(N, 1)
    hT = np.ascontiguousarray(f(h).T)                       # [H, N]
    c = np.ascontiguousarray(f(c))
    enc = f(encoder_hiddens)                                # [N, L, H]
    msk = np.asarray(attn_mask).astype(np.uint8)
    embW = np.ascontiguousarray(f(embed_W))
    wihT = np.ascontiguousarray(f(w_ih).T)                  # [E, 4H]
    whhT = np.ascontiguousarray(f(w_hh).T)                  # [H, 4H]
    bg = (np.asarray(b_ih, np.float64) + np.asarray(b_hh, np.float64)) \
        .astype(np.float32).reshape(1, G4)
    awhT = np.ascontiguousarray(f(attn_Wh).T.astype(bf))
    awsT = np.ascontiguousarray(f(attn_Ws).T.astype(bf))
    av = f(attn_v).reshape(1, H).astype(bf)
    hoWT = f(h2o_W).T.astype(bf)                            # [2H, V]
    hob = f(h2o_b)

    in_maps = []
    for k in range(NCORES):
        sel = np.zeros((N, NB), np.float32)
        sel[np.arange(k * NB, (k + 1) * NB), np.arange(NB)] = 1.0
        encs = enc[k * NB:(k + 1) * NB]                     # [NB, L, H]
        in_maps.append({
            "ids": ids32,
            "hT": hT,
            "c0": c,
            "encN": np.ascontiguousarray(
                encs.transpose(1, 0, 2).astype(bf)),        # [L, NB, H]
            "encT": np.ascontiguousarray(
                encs.reshape(NB * L, H).T.astype(bf)),      # [H, NB*L]
            "msk": np.ascontiguousarray(msk[k * NB:(k + 1) * NB]),
            "embW": embW,
            "wihT": wihT,
            "whhT": whhT,
            "bg": bg,
            "awhT": awhT,
            "awsT": awsT,
            "av": av,
            "howT": np.ascontiguousarray(hoWT[:, k * VS:(k + 1) * VS]),
            "hob": np.ascontiguousarray(hob[k * VS:(k + 1) * VS]).reshape(1, VS),
            "sel": sel,
        })
    return in_maps


def kernel(**inputs):
    from concourse.bass_utils import run_bass_kernel_spmd

    nc = _get_nc()
    in_maps = _make_in_maps(**inputs)
    res = run_bass_kernel_spmd(nc, in_maps, list(range(NCORES)))
    out = np.concatenate([res.results[k]["out_c"] for k in range(NCORES)], axis=1)
    ht = res.results[0]["ht_o"]
    ct = res.results[0]["ct_o"]
    return out, ht, ct
